# revision 29
# baseline (speedup 1.0000x reference)
"""Trainium2 Bass kernel for nn_ConvM_Layer (episode covariance similarity).

Math reformulation (exact):
  cov      = S_c S_c^T / (hw-1)  with S_c the per-(t,way) centered support (c x 500)
  cov_sim  = q^T cov q = ||S_c^T q||^2 / (hw-1)  >= 0   (PSD quadratic form)
  => LeakyReLU is the identity, and
  score[t,q,w] = sum_p conv_w[p]/(hw-1) * ||S_c^T (q_p - qbar)||^2 + conv_b

Sharding: 8 cores = (t in 0..3) x (wq half in 0..1); wq padded 75 -> 76 = 2*38.

Per-core structure:
  - host centers q (over 100 positions) and s (over 500 samples), casts to
    bf16, channel-major; queries pack densely into 3800 (+40 pad) columns
    = 30 tiles (tau) of 128.
  - PE, way-outer: for each way, 30 tau x 5 c-tiles of [128,128] x [128,500]
    matmuls accumulate P = S_c^T Q into PSUM. Way-outer lets compute start
    once way 0's support slice (0.64 MB) lands instead of all of s.
  - PSUM drains are split across engines: ScalarE squares each bank into
    a bf16 scratch tile, VectorE reduce_sums it into one lcs column, so
    neither engine gates the PE's PSUM-bank recycle (~670 + ~320 ns vs
    the 1042 ns PE step).
  - The Conv1d (kernel=stride=100) becomes 30 tiny accumulating matmuls
    with a host-built banded map G[128, 30*38] (w/99 routed per position):
    score[way, q] = sum_tau lcs[:, tau*5:+5]^T @ G[:, tau*38:+38].
"""

from contextlib import ExitStack

import numpy as np
import ml_dtypes

import concourse.bass as bass
import concourse.tile as tile
from concourse import bacc, mybir
from concourse.bass_utils import run_bass_kernel_spmd

# Problem shape (hardcoded per contract)
T, WQ, C, H, W = 4, 75, 640, 10, 10
HW = H * W                 # 100
WAY, SHOT = 5, 5
M = SHOT * HW              # 500 support samples per way
WQP = 76                   # padded query count (divisible by 2)
WQH = WQP // 2             # 38 queries per core
NQ = WQH * HW              # 3800 query spatial columns per core
NT = 30                    # query-position tiles of 128 (3840 = 30*128)
NQP = NT * 128             # padded query columns
CT = C // 128              # 5 contraction tiles
N_CORES = 8
N_WARM = 9                 # dummy matmuls that pre-warm the PE clock gate
SPLIT_T = 3                # leading tau tiles whose ct groups open early
SPLIT_CT = 3               # first ct group size for the early DMAs

F32 = mybir.dt.float32
F32R = mybir.dt.float32r
BF16 = mybir.dt.bfloat16

# q-column DMA chunks (col ranges); sized so chunk k lands before the
# way-0 sweep reaches it
Q_CHUNKS = [
    (0, 256), (256, 512), (512, 768), (768, 1024), (1024, 1536),
    (1536, 2176), (2176, 2816), (2816, NQP),
]

_CACHE: dict = {}


def _kernel_body(ctx: ExitStack, tc: tile.TileContext, q_d, s_d, g_d, o_d):
    nc = tc.nc

    warm_p = ctx.enter_context(tc.tile_pool(name="warm", bufs=1))
    sc_p = ctx.enter_context(tc.tile_pool(name="sc", bufs=1))
    qc_p = ctx.enter_context(tc.tile_pool(name="qc", bufs=1))
    g_p = ctx.enter_context(tc.tile_pool(name="g", bufs=1))
    lcs_p = ctx.enter_context(tc.tile_pool(name="lcs", bufs=1))
    trash_p = ctx.enter_context(tc.tile_pool(name="trash", bufs=3))
    osb_p = ctx.enter_context(tc.tile_pool(name="osb", bufs=1))
    ps_p = ctx.enter_context(tc.tile_pool(name="ps", bufs=6, space="PSUM"))
    wps_p = ctx.enter_context(tc.tile_pool(name="wps", bufs=1, space="PSUM"))
    ops_p = ctx.enter_context(tc.tile_pool(name="ops", bufs=1, space="PSUM"))

    # ---- PE warm-up: matmuls on a mostly-uninitialized tile (garbage is
    # fine — wps is never read); 1-col memset just satisfies alloc-on-write ----
    wsrc = warm_p.tile([128, 512], BF16, name="wsrc")
    nc.gpsimd.memset(wsrc[:, 0:1], 0.0)
    wps = wps_p.tile([128, 512], F32, name="wpsum")
    for _ in range(N_WARM):
        nc.tensor.matmul(wps[:], wsrc[:, :128], wsrc[:], start=True, stop=True)

    # ---- support, centered on host: sc col = ct*2500 + way*500 + m ----
    sc = sc_p.tile([128, CT * WAY * M], BF16, name="sc")
    sc_v = sc.rearrange("p (c w m) -> p c w m", c=CT, w=WAY)
    sd_v = s_d.rearrange("(c p) m -> p c m", p=128)

    # ---- queries, centered on host: qc col = ct*NQP + n ----
    qc = qc_p.tile([128, CT * NQP], BF16, name="qc")
    qc_v = qc.rearrange("p (c n) -> p c n", c=CT)
    qd_v = q_d.rearrange("(c p) n -> p c n", p=128)

    # way-0 support + first q cols split by ct group so the leading
    # accumulation groups can open as soon as the small group-A DMA lands
    k, e0 = SPLIT_CT, Q_CHUNKS[0][1]
    nc.sync.dma_start(sc_v[:, :k, 0, :], sd_v[:, :k, 0:M])
    nc.sync.dma_start(qc_v[:, :k, 0:e0], qd_v[:, :k, 0:e0])
    nc.sync.dma_start(sc_v[:, k:, 0, :], sd_v[:, k:, 0:M])
    nc.sync.dma_start(qc_v[:, k:, 0:e0], qd_v[:, k:, 0:e0])
    for c0, c1 in Q_CHUNKS[1:]:
        nc.sync.dma_start(qc_v[:, :, c0:c1], qd_v[:, :, c0:c1])

    # remaining support ways + conv map G (needed late)
    for wy in range(1, WAY):
        nc.sync.dma_start(
            sc_v[:, :, wy, :], sd_v[:, :, wy * M:(wy + 1) * M]
        )
    g_sb = g_p.tile([128, NT * WQH], F32R, name="gmap")
    nc.sync.dma_start(g_sb[:], g_d[:])

    # per-(tau, way) squared norms of P = S_c^T Q
    lcs = lcs_p.tile([128, NT * WAY], F32R, name="lcs")

    def mm(ps, wy, t, ct):
        nc.tensor.matmul(
            ps,
            qc[:, ct * NQP + t * 128:ct * NQP + (t + 1) * 128],
            sc[:, ct * WAY * M + wy * M:ct * WAY * M + (wy + 1) * M],
            start=(ct == 0),
            stop=(ct == CT - 1),
        )

    # score accumulator [WAY, WQH]; G-matmuls interleave into way-4's sweep
    ops = ops_p.tile([WAY, WQH], F32, name="opsum")

    def gmm(j):
        nc.tensor.matmul(
            ops[:],
            lcs[:, j * WAY:(j + 1) * WAY],
            g_sb[:, j * WQH:(j + 1) * WQH],
            start=(j == 0),
            stop=(j == NT - 1),
        )

    # ---- main: P tile [128 pos, 500 m] per (way, tau); lcs col = ||.||^2 ----
    # head: open SPLIT_T groups with just group-A c-tiles (early DMA)
    head_ps = [ps_p.tile([128, M], F32, name="ps", tag="ps")
               for _ in range(SPLIT_T)]
    for t in range(SPLIT_T):
        for ct in range(SPLIT_CT):
            mm(head_ps[t], 0, t, ct)

    for wy in range(WAY):
        for t in range(NT):
            if wy == 0 and t < SPLIT_T:
                ps = head_ps[t]
                for ct in range(SPLIT_CT, CT):
                    mm(ps, wy, t, ct)
            else:
                ps = ps_p.tile([128, M], F32, name="ps", tag="ps")
                for ct in range(CT):
                    mm(ps, wy, t, ct)
            col = t * WAY + wy
            with nc.allow_low_precision(reason="f32r out is full fp32 bits"):
                trash = trash_p.tile([128, M], BF16, tag="tr", name="tra")
                if wy == WAY - 1 and t >= NT - 2:
                    # shortest serial tail: single fused square+accum on ACT
                    nc.scalar.activation(
                        trash[:], ps, mybir.ActivationFunctionType.Square,
                        accum_out=lcs[:, col:col + 1],
                    )
                else:
                    nc.scalar.activation(
                        trash[:], ps, mybir.ActivationFunctionType.Square,
                    )
                    nc.vector.reduce_sum(
                        lcs[:, col:col + 1], trash[:],
                        axis=mybir.AxisListType.X,
                    )
            if wy == WAY - 1 and t >= 2:
                gmm(t - 2)   # lag-2: its lcs columns drained ~2 steps ago

    # ---- remaining score accumulation tail ----
    gmm(NT - 2)
    gmm(NT - 1)
    osb = osb_p.tile([WAY, WQH], F32, name="osb")
    nc.scalar.copy(osb[:], ops[:])
    nc.sync.dma_start(o_d[:], osb[:])


def _build():
    key = "nc"
    if key in _CACHE:
        return _CACHE[key]
    nc = bacc.Bacc(
        "TRN2", target_bir_lowering=False, debug=False, num_devices=N_CORES
    )
    q_d = nc.dram_tensor("q", [C, NQP], BF16, kind="ExternalInput").ap()
    s_d = nc.dram_tensor("s", [C, WAY * M], BF16, kind="ExternalInput").ap()
    g_d = nc.dram_tensor("g", [128, NT * WQH], F32R, kind="ExternalInput").ap()
    o_d = nc.dram_tensor("out", [WAY, WQH], F32, kind="ExternalOutput").ap()
    with tile.TileContext(nc) as tc:
        with ExitStack() as ctx:
            _kernel_body(ctx, tc, q_d, s_d, g_d, o_d)
    nc.compile()
    _CACHE[key] = nc
    return nc


def make_in_maps(query_feat, support_feat, conv_w):
    q = np.asarray(query_feat, dtype=np.float32).reshape(T, WQ, C, HW)
    s = np.asarray(support_feat, dtype=np.float32).reshape(T, WAY, SHOT, C, HW)
    w = np.asarray(conv_w, dtype=np.float32)[0, 0] / (HW - 1)

    # center on host (f32), then channel-major + bf16
    q = q - q.mean(axis=3, keepdims=True)                    # (T, WQ, C, HW)
    qt = np.zeros((T, C, WQP * HW), dtype=np.float32)
    qt[:, :, :WQ * HW] = q.transpose(0, 2, 1, 3).reshape(T, C, WQ * HW)

    s = s.transpose(0, 1, 3, 2, 4).reshape(T, WAY, C, M)     # (T, WAY, C, 500)
    s = s - s.mean(axis=3, keepdims=True)
    st = np.ascontiguousarray(
        s.transpose(0, 2, 1, 3).reshape(T, C, WAY * M)
    ).astype(ml_dtypes.bfloat16)

    # banded conv map: G[p, tau*WQH + q] = w[h]  at n = tau*128+p = q*100+h
    g = np.zeros((128, NT * WQH), dtype=np.float32)
    n = np.arange(NQ)
    g[n % 128, (n // 128) * WQH + (n // HW)] = w[n % HW]

    in_maps = []
    for core in range(N_CORES):
        ti, half = core // 2, core % 2
        qh = np.zeros((C, NQP), dtype=np.float32)
        qh[:, :NQ] = qt[ti, :, half * NQ:(half + 1) * NQ]
        in_maps.append({
            "q": qh.astype(ml_dtypes.bfloat16),
            "s": st[ti],
            "g": g,
        })
    return in_maps


LAST_RESULT = None  # set by kernel(); lets a harness read exec_time_ns/profile


def kernel(query_feat, support_feat, conv_w, conv_b):
    global LAST_RESULT
    nc = _build()
    in_maps = make_in_maps(query_feat, support_feat, conv_w)
    res = run_bass_kernel_spmd(nc, in_maps, list(range(N_CORES)))
    LAST_RESULT = res
    score = np.empty((T, WQP, WAY), dtype=np.float32)
    for core in range(N_CORES):
        ti, half = core // 2, core % 2
        blk = res.results[core]["out"]  # [WAY, WQH]
        score[ti, half * WQH:(half + 1) * WQH, :] = blk.T
    out = score[:, :WQ, :] + np.asarray(conv_b, dtype=np.float32)[0]
    return np.ascontiguousarray(out)
